# revision 33
# baseline (speedup 1.0000x reference)
"""v4 reconstruction: mexp mask-fold variant (measured 99011 ns earlier)."""

import os

import numpy as np

import concourse.mybir as mybir
from concourse import bacc
from concourse.bass_utils import run_bass_kernel_spmd
from concourse.tile import TileContext

B, D, H, F, E, TOPK = 32768, 1024, 512, 100, 16, 2
N_CORES = 8
BL = B // N_CORES
PS = 512
NPOS = BL // PS
MH = H // 128
KD = D // 128
KH = H // 128
NWARM = 32
COMPUTE_DT = "bfloat16"

_FP32 = mybir.dt.float32
_cache = {}


def _np_in_dtype():
    import ml_dtypes

    return ml_dtypes.bfloat16


def _geom(pattern):
    kt3 = [(ec * F + 127) // 128 for ec in pattern]
    col0, c = [], 0
    for ec in pattern:
        col0.append(c)
        c += ec * F
    w1w = max(col0[p] + kt3[p] * 128 for p in range(len(pattern)))
    boff, b = [], 0
    for k in kt3:
        boff.append(b)
        b += k
    return kt3, col0, w1w, boff, b


def _build_nc(key):
    pattern, zero_bias, zero_b2 = key
    CDT = getattr(mybir.dt, COMPUTE_DT)
    kt3, col0, W1W, boff, SKT = _geom(pattern)
    NA = boff[NPOS // 2]

    nc = bacc.Bacc("TRN2", target_bir_lowering=False, num_devices=N_CORES)

    xT_d = nc.declare_dram_parameter("xT", [D * BL], CDT, isOutput=False)
    wsh_d = nc.declare_dram_parameter("wsh", [128, KD * H], CDT, isOutput=False)
    w1_d = nc.declare_dram_parameter("w1all", [H, W1W], CDT, isOutput=False)
    mexp_d = nc.declare_dram_parameter("mexp", [128, SKT * PS], CDT, isOutput=False)
    w2_d = nc.declare_dram_parameter("w2bd", [128, SKT], CDT, isOutput=False)
    out_d = nc.declare_dram_parameter("out", [BL], _FP32, isOutput=True)

    relu = mybir.ActivationFunctionType.Relu

    with TileContext(nc) as tc:
        with (
            tc.tile_pool(name="weights", bufs=1) as wpool,
            tc.tile_pool(name="xin", bufs=1) as xpool,
            tc.tile_pool(name="mid", bufs=4) as midpool,
            tc.tile_pool(name="act", bufs=2) as apool,
            tc.tile_pool(name="small", bufs=3) as spool,
            tc.tile_pool(name="ps_h", bufs=4, space="PSUM") as ps_h,
            tc.tile_pool(name="ps_a", bufs=2, space="PSUM") as ps_a,
            tc.tile_pool(name="ps_o", bufs=2, space="PSUM") as ps_o,
        ):
            _prio = [0]

            def pdma(q, dst, src):
                inst = q.dma_start(dst, src)
                inst.ins.bass_priority = _prio[0]
                _prio[0] += 1
                return inst

            warm_sb = wpool.tile([128, 128], CDT, name="warm")
            nc.vector.memset(warm_sb[:], 0.0)

            def xview(s):
                o = s * PS * D
                return xT_d[o : o + PS * D].rearrange(
                    "(p ko t) -> p ko t", p=128, t=PS
                )

            wsh_view = wsh_d.rearrange("p (o h) -> p o h", h=H)
            wsh_ks = [wpool.tile([128, H], CDT, name=f"wshk{k}") for k in range(KD)]
            x0v = xview(0)
            x0 = [xpool.tile([128, PS], CDT, name=f"x0_{k}") for k in range(KD)]

            for k in range(0, KD, 2):
                pdma(nc.sync, wsh_ks[k][:], wsh_view[:, k])
                pdma(nc.sync, x0[k][:], x0v[:, k])
                pdma(nc.scalar, wsh_ks[k + 1][:], wsh_view[:, k + 1])
                pdma(nc.scalar, x0[k + 1][:], x0v[:, k + 1])

            def wsh_k(k, m):
                return wsh_ks[k][:, m * 128 : (m + 1) * 128]

            xs = [None] * NPOS
            w1_view = w1_d.rearrange("(o p) f -> p o f", p=128)
            w1_ks = [wpool.tile([128, W1W], CDT, name=f"w1k{k}") for k in range(KH)]
            mexp_view = mexp_d.rearrange("p (s t) -> p s t", t=PS)
            mexp_a = wpool.tile([128, NA, PS], CDT, name="mexp_a")
            mexp_b = wpool.tile([128, SKT - NA, PS], CDT, name="mexp_b")
            w2_sb = wpool.tile([128, SKT], CDT, name="w2bd")

            def xdma(s, halves=False):
                if halves:
                    # two tiles so M1 can start on the first half
                    h = KD // 2
                    xa = xpool.tile([128, h, PS], CDT, name=f"xs{s}a")
                    xb = xpool.tile([128, KD - h, PS], CDT, name=f"xs{s}b")
                    pdma(nc.sync, xa[:], xview(s)[:, :h])
                    pdma(nc.sync, xb[:], xview(s)[:, h:])
                    xs[s] = (xa, xb)
                else:
                    xs[s] = xpool.tile([128, KD, PS], CDT, name=f"xs{s}")
                    pdma(nc.sync, xs[s][:], xview(s))

            xdma(1, halves=True)
            xdma(2, halves=True)
            pdma(nc.sync, w1_ks[0][:], w1_view[:, 0])
            pdma(nc.sync, w1_ks[1][:], w1_view[:, 1])
            pdma(nc.scalar, w1_ks[2][:], w1_view[:, 2])
            pdma(nc.scalar, w1_ks[3][:], w1_view[:, 3])
            pdma(nc.sync, mexp_a[:], mexp_view[:, :NA])
            pdma(nc.sync, w2_sb[:], w2_d[:])
            xdma(3)
            pdma(nc.sync, mexp_b[:], mexp_view[:, NA:])
            for s in range(4, NPOS):
                xdma(s)

            pw = ps_h.tile([128, PS], _FP32, tag="ps_h", name="pwarm")
            for _ in range(NWARM):
                nc.tensor.matmul(
                    pw[:, :128], lhsT=warm_sb[:], rhs=warm_sb[:],
                    start=True, stop=True,
                )

            hTs = [None] * NPOS

            def m1_chunk(s):
                hT = midpool.tile([128, MH, PS], CDT, tag="hT", name=f"hT{s}")
                hTs[s] = hT
                if s == 0:
                    phs = [
                        ps_h.tile([128, PS], _FP32, tag="ps_h", name=f"ph0_{m}")
                        for m in range(MH)
                    ]
                    for k in range(KD):
                        for m in range(MH):
                            nc.tensor.matmul(
                                phs[m][:], lhsT=wsh_k(k, m), rhs=x0[k][:],
                                start=(k == 0), stop=(k == KD - 1),
                            )
                    for m in range(MH):
                        nc.scalar.activation(hT[:, m, :], phs[m][:], relu)
                else:
                    def xsl(k):
                        t = xs[s]
                        if isinstance(t, tuple):
                            h = KD // 2
                            return t[0][:, k, :] if k < h else t[1][:, k - h, :]
                        return t[:, k, :]

                    for m in range(MH):
                        ph = ps_h.tile(
                            [128, PS], _FP32, tag="ps_h", name=f"ph{s}_{m}"
                        )
                        for k in range(KD):
                            nc.tensor.matmul(
                                ph[:], lhsT=wsh_k(k, m), rhs=xsl(k),
                                start=(k == 0), stop=(k == KD - 1),
                            )
                        nc.scalar.activation(hT[:, m, :], ph[:], relu)

            ams = [None] * NPOS

            def m2(p):
                hT = hTs[p]
                kt = kt3[p]
                if boff[p] < NA:
                    mx, mo = mexp_a, boff[p]
                else:
                    mx, mo = mexp_b, boff[p] - NA
                aT = apool.tile([128, kt, PS], CDT, tag="aT", name=f"aT{p}")
                am = apool.tile([128, kt, PS], CDT, tag="am", name=f"am{p}")
                ams[p] = am
                for m in range(kt):
                    pa = ps_a.tile([128, PS], _FP32, tag="ps_a", name=f"pa{p}_{m}")
                    c0 = col0[p] + m * 128
                    for k in range(KH):
                        nc.tensor.matmul(
                            pa[:], lhsT=w1_ks[k][:, c0 : c0 + 128],
                            rhs=hT[:, k, :],
                            start=(k == 0), stop=(k == KH - 1),
                        )
                    nc.scalar.activation(aT[:, m, :], pa[:], relu)
                    nc.vector.tensor_mul(
                        am[:, m, :], aT[:, m, :], mx[:, mo + m, :]
                    )

            def m3out(p):
                kt = kt3[p]
                am = ams[p]
                po = ps_o.tile([1, PS], _FP32, tag="ps_o", name=f"po{p}")
                for k in range(kt):
                    nc.tensor.matmul(
                        po[:],
                        lhsT=w2_sb[:, boff[p] + k : boff[p] + k + 1],
                        rhs=am[:, k, :],
                        start=(k == 0), stop=(k == kt - 1),
                    )
                g0 = p * PS
                ot = spool.tile([1, PS], _FP32, tag="ot", name=f"ot{p}")
                nc.vector.tensor_copy(ot[:], po[:])
                nc.gpsimd.dma_start(
                    out_d[g0 : g0 + PS].rearrange("(o t) -> o t", o=1), ot[:]
                )

            m1_chunk(0)
            m1_chunk(1)
            m1_chunk(2)
            for p in range(NPOS):
                m2(p)
                if p >= 1:
                    m3out(p - 1)
                if p + 3 < NPOS:
                    m1_chunk(p + 3)
            m3out(NPOS - 1)

    nc.compile()
    return nc


def get_nc(key):
    ckey = (COMPUTE_DT, key)
    if ckey not in _cache:
        _cache[ckey] = _build_nc(key)
    return _cache[ckey]


def prepare(inputs):
    np_dt = _np_in_dtype()
    x = np.asarray(inputs["x"], dtype=np.float32)
    idx = np.asarray(inputs["idx"]).astype(np.int64).reshape(B)
    W_shared = np.asarray(inputs["W_shared"], dtype=np.float32)
    W1 = np.asarray(inputs["W1"], dtype=np.float32)
    W2 = np.asarray(inputs["W2"], dtype=np.float32).reshape(E, F)
    send_to = np.asarray(inputs["send_to"]).astype(np.int64)

    perm = np.argsort(idx, kind="stable")
    idx_s = idx[perm]
    routes_s = send_to[idx_s]
    x_s = x[perm]

    NCH = B // PS
    chex = [np.unique(routes_s[g * PS : (g + 1) * PS]) for g in range(NCH)]
    order = np.argsort([-len(e) for e in chex], kind="stable")
    pattern = tuple(
        int(max(len(chex[order[p * N_CORES + c]]) for c in range(N_CORES)))
        for p in range(NPOS)
    )
    kt3, col0, W1W, boff, SKT = _geom(pattern)

    wsh = np.ascontiguousarray(
        W_shared.reshape(KD, 128, H).transpose(1, 0, 2).reshape(128, KD * H)
    ).astype(np_dt)

    key = (pattern, True, True)
    in_maps = []
    tok_order_parts = []
    for c in range(N_CORES):
        w1all = np.zeros((H, W1W), dtype=np.float32)
        mexp = np.zeros((128, SKT, PS), dtype=np.float32)
        w2bd = np.zeros((128, SKT), dtype=np.float32)
        xparts = []
        for p in range(NPOS):
            g = order[p * N_CORES + c]
            toks = perm[g * PS : (g + 1) * PS]
            tok_order_parts.append(toks)
            xc = x_s[g * PS : (g + 1) * PS]
            xparts.append(
                np.ascontiguousarray(xc.T)
                .reshape(KD, 128, PS)
                .transpose(1, 0, 2)
                .ravel()
            )
            r = routes_s[g * PS : (g + 1) * PS]
            el = chex[g]
            ecp, kt = pattern[p], kt3[p]
            slots = np.full(ecp, -1, dtype=np.int64)
            slots[: len(el)] = el

            w2blk = np.zeros(kt * 128, dtype=np.float32)
            for j, e in enumerate(slots):
                if e < 0:
                    continue
                w1all[:, col0[p] + j * F : col0[p] + (j + 1) * F] = W1[e]
                w2blk[j * F : (j + 1) * F] = W2[e]
            w2bd[:, boff[p] : boff[p] + kt] = w2blk.reshape(kt, 128).T

            mrow = np.zeros((ecp, PS), dtype=np.float32)
            for kk in range(r.shape[1]):
                mrow += (slots[:, None] == r[None, :, kk]) / r.shape[1]
            mflat = np.zeros((kt * 128, PS), dtype=np.float32)
            for j in range(ecp):
                mflat[j * F : (j + 1) * F] = mrow[j]
            mexp[:, boff[p] : boff[p] + kt, :] = (
                mflat.reshape(kt, 128, PS).transpose(1, 0, 2)
            )

        xT = np.concatenate(xparts).astype(np_dt)
        in_maps.append(
            {
                "xT": xT,
                "wsh": wsh,
                "w1all": w1all.astype(np_dt),
                "mexp": np.ascontiguousarray(
                    mexp.reshape(128, SKT * PS)
                ).astype(np_dt),
                "w2bd": w2bd.astype(np_dt),
            }
        )
    tok_order = np.concatenate(tok_order_parts)
    return key, in_maps, tok_order


def kernel(**inputs) -> np.ndarray:
    key, in_maps, tok_order = prepare(inputs)
    nc = get_nc(key)
    res = run_bass_kernel_spmd(nc, in_maps, list(range(N_CORES)))
    out_sorted = np.concatenate([res.results[c]["out"] for c in range(N_CORES)])
    out = np.empty(B, dtype=np.float32)
    out[tok_order] = out_sorted
    return out.reshape(B, 1)


# revision 35
# speedup vs baseline: 1.0107x; 1.0107x over previous
"""Trainium2 Bass kernel for the MoE-routing module.

Computation (B=32768, D=1024, H=512, F=100, E=16, K=2):
    h   = relu(x @ W_shared)                             [B, H]
    a   = relu(einsum('bh,ehf', h, W1))                  [B, E, F]
    o   = einsum('bef,efo', a, W2)                       [B, E, 1]
    out = mean over the K routed experts of o[b, send_to[idx[b]]]
(The reference's biases are structurally zero and are elided.)

Strategy:
  * Host sorts tokens by head id, cuts the sorted batch into 64 chunks of
    512 tokens.  A chunk touches 2 experts (3 when it straddles a head-id
    boundary; there are <=15 such chunks).  Chunks are dealt to the 8
    cores so every core sees the same per-position expert-slot pattern
    (typically [3,3,2,2,2,2,2,2]) -> one SPMD program, data-parallel over
    the batch, no collectives (outputs are disjoint row shards).
  * Per-core compute, features on SBUF partitions throughout (bf16):
      M1: hT[h, t]  = relu(W_shared.T @ xT)    512-token chunks
      M2: aT[f',t]  = relu(W1sel.T @ hT)       f' = chunk-local slot*F + f
      mask fold:  am = aT * mexp   (routing mask expanded to f' rows)
      M3: out[t]    = sum_k w2col_k.T @ am_k   (single-partition result)
    Folding the mask into the activations removes the select-matmul and
    its PSUM bank: PSUM = 4 (M1) + 2 (M2) + 2 (M3) banks.
  * bf16 operands: half the HBM bytes of fp32, same PE column rate, and
    FWL (disabled for fp32) helps hide LDWEIGHTS.
  * All DMA sources are partition-major (128 contiguous multi-KB lines
    per descriptor -- cheap to issue); the Sync queue carries the bulk
    x/w1/mask stream in first-need order (x chunks 1-2 split in halves so
    M1 can start on half a chunk), the Scalar queue carries only small
    early pieces and then RELUs (per-queue streams are reordered by
    dependency-readiness, so big dep-free transfers must never share a
    queue with compute), GpSimd carries the dependency-gated out DMAs.
  * ~32 warm-up matmuls run during the fixed ~7us runtime preamble so
    the PE HAM clock gate is at 2.4 GHz when real work arrives; M1 runs
    three chunks ahead of M2 and each M3 trails its M2 by one position,
    so the tensor queue never head-blocks on the scalar/vector chain.

Measured: 98.0us (baseline 132.9us); max rel err 4.3e-3 (gate 2e-2).
"""

import os

import numpy as np

import concourse.mybir as mybir
from concourse import bacc
from concourse.bass_utils import run_bass_kernel_spmd
from concourse.tile import TileContext

B, D, H, F, E, TOPK = 32768, 1024, 512, 100, 16, 2
N_CORES = 8
BL = B // N_CORES
PS = 512
NPOS = BL // PS
MH = H // 128
KD = D // 128
KH = H // 128
NWARM = 32
COMPUTE_DT = "bfloat16"

_FP32 = mybir.dt.float32
_cache = {}


def _np_in_dtype():
    import ml_dtypes

    return ml_dtypes.bfloat16


def _geom(pattern):
    kt3 = [(ec * F + 127) // 128 for ec in pattern]
    col0, c = [], 0
    for ec in pattern:
        col0.append(c)
        c += ec * F
    w1w = max(col0[p] + kt3[p] * 128 for p in range(len(pattern)))
    boff, b = [], 0
    for k in kt3:
        boff.append(b)
        b += k
    return kt3, col0, w1w, boff, b


def _build_nc(key):
    pattern, zero_bias, zero_b2 = key
    CDT = getattr(mybir.dt, COMPUTE_DT)
    kt3, col0, W1W, boff, SKT = _geom(pattern)
    NA = boff[NPOS // 2]

    nc = bacc.Bacc("TRN2", target_bir_lowering=False, num_devices=N_CORES)

    xT_d = nc.declare_dram_parameter("xT", [D * BL], CDT, isOutput=False)
    wsh_d = nc.declare_dram_parameter("wsh", [128, KD * H], CDT, isOutput=False)
    w1_d = nc.declare_dram_parameter("w1all", [H, W1W], CDT, isOutput=False)
    mexp_d = nc.declare_dram_parameter("mexp", [128, SKT * PS], CDT, isOutput=False)
    w2_d = nc.declare_dram_parameter("w2bd", [128, SKT], CDT, isOutput=False)
    out_d = nc.declare_dram_parameter("out", [BL], _FP32, isOutput=True)

    relu = mybir.ActivationFunctionType.Relu

    with TileContext(nc) as tc:
        with (
            tc.tile_pool(name="weights", bufs=1) as wpool,
            tc.tile_pool(name="xin", bufs=1) as xpool,
            tc.tile_pool(name="mid", bufs=4) as midpool,
            tc.tile_pool(name="act", bufs=2) as apool,
            tc.tile_pool(name="small", bufs=3) as spool,
            tc.tile_pool(name="ps_h", bufs=4, space="PSUM") as ps_h,
            tc.tile_pool(name="ps_a", bufs=2, space="PSUM") as ps_a,
            tc.tile_pool(name="ps_o", bufs=2, space="PSUM") as ps_o,
        ):
            _prio = [0]

            def pdma(q, dst, src):
                inst = q.dma_start(dst, src)
                inst.ins.bass_priority = _prio[0]
                _prio[0] += 1
                return inst

            warm_sb = wpool.tile([128, 128], CDT, name="warm")
            nc.vector.memset(warm_sb[:], 0.0)

            def xview(s):
                o = s * PS * D
                return xT_d[o : o + PS * D].rearrange(
                    "(p ko t) -> p ko t", p=128, t=PS
                )

            wsh_view = wsh_d.rearrange("p (o h) -> p o h", h=H)
            wsh_ks = [wpool.tile([128, H], CDT, name=f"wshk{k}") for k in range(KD)]
            x0v = xview(0)
            x0 = [xpool.tile([128, PS], CDT, name=f"x0_{k}") for k in range(KD)]

            for k in range(0, KD, 2):
                pdma(nc.sync, wsh_ks[k][:], wsh_view[:, k])
                pdma(nc.sync, x0[k][:], x0v[:, k])
                pdma(nc.scalar, wsh_ks[k + 1][:], wsh_view[:, k + 1])
                pdma(nc.scalar, x0[k + 1][:], x0v[:, k + 1])

            def wsh_k(k, m):
                return wsh_ks[k][:, m * 128 : (m + 1) * 128]

            xs = [None] * NPOS
            w1_view = w1_d.rearrange("(o p) f -> p o f", p=128)
            w1_ks = [wpool.tile([128, W1W], CDT, name=f"w1k{k}") for k in range(KH)]
            mexp_view = mexp_d.rearrange("p (s t) -> p s t", t=PS)
            mexp_a = wpool.tile([128, NA, PS], CDT, name="mexp_a")
            mexp_b = wpool.tile([128, SKT - NA, PS], CDT, name="mexp_b")
            w2_sb = wpool.tile([128, SKT], CDT, name="w2bd")

            def xdma(s, halves=False):
                if halves:
                    # two tiles so M1 can start on the first half
                    h = KD // 2
                    xa = xpool.tile([128, h, PS], CDT, name=f"xs{s}a")
                    xb = xpool.tile([128, KD - h, PS], CDT, name=f"xs{s}b")
                    pdma(nc.sync, xa[:], xview(s)[:, :h])
                    pdma(nc.sync, xb[:], xview(s)[:, h:])
                    xs[s] = (xa, xb)
                else:
                    xs[s] = xpool.tile([128, KD, PS], CDT, name=f"xs{s}")
                    pdma(nc.sync, xs[s][:], xview(s))

            xdma(1, halves=True)
            xdma(2, halves=True)
            pdma(nc.sync, w1_ks[0][:], w1_view[:, 0])
            pdma(nc.sync, w1_ks[1][:], w1_view[:, 1])
            pdma(nc.sync, w1_ks[2][:], w1_view[:, 2])
            pdma(nc.sync, w1_ks[3][:], w1_view[:, 3])
            pdma(nc.sync, mexp_a[:], mexp_view[:, :NA])
            pdma(nc.sync, w2_sb[:], w2_d[:])
            xdma(3)
            pdma(nc.sync, mexp_b[:], mexp_view[:, NA:])
            for s in range(4, NPOS):
                xdma(s)

            pw = ps_h.tile([128, PS], _FP32, tag="ps_h", name="pwarm")
            for _ in range(NWARM):
                nc.tensor.matmul(
                    pw[:, :128], lhsT=warm_sb[:], rhs=warm_sb[:],
                    start=True, stop=True,
                )

            hTs = [None] * NPOS

            def m1_chunk(s):
                hT = midpool.tile([128, MH, PS], CDT, tag="hT", name=f"hT{s}")
                hTs[s] = hT
                if s == 0:
                    phs = [
                        ps_h.tile([128, PS], _FP32, tag="ps_h", name=f"ph0_{m}")
                        for m in range(MH)
                    ]
                    for k in range(KD):
                        for m in range(MH):
                            nc.tensor.matmul(
                                phs[m][:], lhsT=wsh_k(k, m), rhs=x0[k][:],
                                start=(k == 0), stop=(k == KD - 1),
                            )
                    for m in range(MH):
                        nc.scalar.activation(hT[:, m, :], phs[m][:], relu)
                else:
                    def xsl(k):
                        t = xs[s]
                        if isinstance(t, tuple):
                            h = KD // 2
                            return t[0][:, k, :] if k < h else t[1][:, k - h, :]
                        return t[:, k, :]

                    for m in range(MH):
                        ph = ps_h.tile(
                            [128, PS], _FP32, tag="ps_h", name=f"ph{s}_{m}"
                        )
                        for k in range(KD):
                            nc.tensor.matmul(
                                ph[:], lhsT=wsh_k(k, m), rhs=xsl(k),
                                start=(k == 0), stop=(k == KD - 1),
                            )
                        nc.scalar.activation(hT[:, m, :], ph[:], relu)

            ams = [None] * NPOS

            def m2(p):
                hT = hTs[p]
                kt = kt3[p]
                if boff[p] < NA:
                    mx, mo = mexp_a, boff[p]
                else:
                    mx, mo = mexp_b, boff[p] - NA
                aT = apool.tile([128, kt, PS], CDT, tag="aT", name=f"aT{p}")
                am = apool.tile([128, kt, PS], CDT, tag="am", name=f"am{p}")
                ams[p] = am
                for m in range(kt):
                    pa = ps_a.tile([128, PS], _FP32, tag="ps_a", name=f"pa{p}_{m}")
                    c0 = col0[p] + m * 128
                    for k in range(KH):
                        nc.tensor.matmul(
                            pa[:], lhsT=w1_ks[k][:, c0 : c0 + 128],
                            rhs=hT[:, k, :],
                            start=(k == 0), stop=(k == KH - 1),
                        )
                    nc.scalar.activation(aT[:, m, :], pa[:], relu)
                    nc.vector.tensor_mul(
                        am[:, m, :], aT[:, m, :], mx[:, mo + m, :]
                    )

            def m3out(p):
                kt = kt3[p]
                am = ams[p]
                po = ps_o.tile([1, PS], _FP32, tag="ps_o", name=f"po{p}")
                for k in range(kt):
                    nc.tensor.matmul(
                        po[:],
                        lhsT=w2_sb[:, boff[p] + k : boff[p] + k + 1],
                        rhs=am[:, k, :],
                        start=(k == 0), stop=(k == kt - 1),
                    )
                g0 = p * PS
                ot = spool.tile([1, PS], _FP32, tag="ot", name=f"ot{p}")
                nc.vector.tensor_copy(ot[:], po[:])
                nc.gpsimd.dma_start(
                    out_d[g0 : g0 + PS].rearrange("(o t) -> o t", o=1), ot[:]
                )

            m1_chunk(0)
            m1_chunk(1)
            m1_chunk(2)
            for p in range(NPOS):
                m2(p)
                if p >= 1:
                    m3out(p - 1)
                if p + 3 < NPOS:
                    m1_chunk(p + 3)
            m3out(NPOS - 1)

    nc.compile()
    return nc


def get_nc(key):
    ckey = (COMPUTE_DT, key)
    if ckey not in _cache:
        _cache[ckey] = _build_nc(key)
    return _cache[ckey]


def prepare(inputs):
    np_dt = _np_in_dtype()
    x = np.asarray(inputs["x"], dtype=np.float32)
    idx = np.asarray(inputs["idx"]).astype(np.int64).reshape(B)
    W_shared = np.asarray(inputs["W_shared"], dtype=np.float32)
    W1 = np.asarray(inputs["W1"], dtype=np.float32)
    W2 = np.asarray(inputs["W2"], dtype=np.float32).reshape(E, F)
    send_to = np.asarray(inputs["send_to"]).astype(np.int64)

    perm = np.argsort(idx, kind="stable")
    idx_s = idx[perm]
    routes_s = send_to[idx_s]
    x_s = x[perm]

    NCH = B // PS
    chex = [np.unique(routes_s[g * PS : (g + 1) * PS]) for g in range(NCH)]
    order = np.argsort([-len(e) for e in chex], kind="stable")
    pattern = tuple(
        int(max(len(chex[order[p * N_CORES + c]]) for c in range(N_CORES)))
        for p in range(NPOS)
    )
    kt3, col0, W1W, boff, SKT = _geom(pattern)

    wsh = np.ascontiguousarray(
        W_shared.reshape(KD, 128, H).transpose(1, 0, 2).reshape(128, KD * H)
    ).astype(np_dt)

    key = (pattern, True, True)
    in_maps = []
    tok_order_parts = []
    for c in range(N_CORES):
        w1all = np.zeros((H, W1W), dtype=np.float32)
        mexp = np.zeros((128, SKT, PS), dtype=np.float32)
        w2bd = np.zeros((128, SKT), dtype=np.float32)
        xparts = []
        for p in range(NPOS):
            g = order[p * N_CORES + c]
            toks = perm[g * PS : (g + 1) * PS]
            tok_order_parts.append(toks)
            xc = x_s[g * PS : (g + 1) * PS]
            xparts.append(
                np.ascontiguousarray(xc.T)
                .reshape(KD, 128, PS)
                .transpose(1, 0, 2)
                .ravel()
            )
            r = routes_s[g * PS : (g + 1) * PS]
            el = chex[g]
            ecp, kt = pattern[p], kt3[p]
            slots = np.full(ecp, -1, dtype=np.int64)
            slots[: len(el)] = el

            w2blk = np.zeros(kt * 128, dtype=np.float32)
            for j, e in enumerate(slots):
                if e < 0:
                    continue
                w1all[:, col0[p] + j * F : col0[p] + (j + 1) * F] = W1[e]
                w2blk[j * F : (j + 1) * F] = W2[e]
            w2bd[:, boff[p] : boff[p] + kt] = w2blk.reshape(kt, 128).T

            mrow = np.zeros((ecp, PS), dtype=np.float32)
            for kk in range(r.shape[1]):
                mrow += (slots[:, None] == r[None, :, kk]) / r.shape[1]
            mflat = np.zeros((kt * 128, PS), dtype=np.float32)
            for j in range(ecp):
                mflat[j * F : (j + 1) * F] = mrow[j]
            mexp[:, boff[p] : boff[p] + kt, :] = (
                mflat.reshape(kt, 128, PS).transpose(1, 0, 2)
            )

        xT = np.concatenate(xparts).astype(np_dt)
        in_maps.append(
            {
                "xT": xT,
                "wsh": wsh,
                "w1all": w1all.astype(np_dt),
                "mexp": np.ascontiguousarray(
                    mexp.reshape(128, SKT * PS)
                ).astype(np_dt),
                "w2bd": w2bd.astype(np_dt),
            }
        )
    tok_order = np.concatenate(tok_order_parts)
    return key, in_maps, tok_order


def kernel(**inputs) -> np.ndarray:
    key, in_maps, tok_order = prepare(inputs)
    nc = get_nc(key)
    res = run_bass_kernel_spmd(nc, in_maps, list(range(N_CORES)))
    out_sorted = np.concatenate([res.results[c]["out"] for c in range(N_CORES)])
    out = np.empty(B, dtype=np.float32)
    out[tok_order] = out_sorted
    return out.reshape(B, 1)
